# revision 5
# baseline (speedup 1.0000x reference)
"""Trainium2 Bass kernel for e3nn-style BatchNorm (instance norm over graphs).

Problem: x [200000, 480] f32, irreps 128x0e + 64x1o + 32x2e, batch_id sorted
into 64 graphs, weight [224], bias [128].

Math (per graph g, derived from the reference):
  scalar block (cols 0:128, one col per channel c):
    m[g,c]   = mean_g(x_c)
    var[g,c] = mean_g(x_c^2) - m^2
    A[g,c]   = w_c / sqrt(var + eps);  B[g,c] = bias_c - m*A
    out      = x*A + B
  vector blocks (64 chans x dim 3, 32 chans x dim 5):
    fn[g,j]  = mean_g(mean_d(x^2))  = (1/d) * sum_d mean_g(x_jd^2)
    A[g,j]   = w_j / sqrt(fn + eps);  out = x*A

Sharding: 8 graphs per core (graph-aligned, boundaries via searchsorted on the
host), each core's rows padded to a common N_pad.  All stats are local to the
core -> no collectives.  Two passes over x per core:
  phase 1: one-hot matmul segment sums (sum x^2 all cols, sum x scalar cols,
           counts) accumulated in PSUM per group, drained to SBUF accums.
  phase 2: gather per-graph params back to rows with a tiny K=8 matmul, then
           fused affine apply in-place, write out.
The local graph id and a ones column ride along as columns 480/481 of the
input so the per-row ids land on partitions with the same DMA as x.
"""

import sys

if "/opt/trn_rl_repo" not in sys.path:
    sys.path.insert(0, "/opt/trn_rl_repo")

import numpy as np

P = 128          # partitions / rows per subtile
KSUB = 8         # subtiles per group
GROUP = P * KSUB # rows per group (1024)
C = 480          # data columns
CW = C + 2       # + local graph id col + ones col
NCORES = 8
G = 64           # total graphs
GPC = G // NCORES  # graphs per core
EPS = 1e-5
R_CACHE = 4      # trailing groups kept resident in SBUF between phases
XT_BUFS = 4

_prog_cache = {}


def _expand(ap_in, rep, bass):
    """Append a trailing broadcast dim [0, rep] to an AP."""
    return bass.AP(tensor=ap_in.tensor, offset=ap_in.offset,
                   ap=[*ap_in.ap, [0, rep]])


def _build(n_pad):
    import concourse.bacc as bacc
    import concourse.bass as bass
    import concourse.tile as tile
    from concourse import mybir

    f32 = mybir.dt.float32
    Alu = mybir.AluOpType
    Act = mybir.ActivationFunctionType

    ng = n_pad // GROUP
    r_cache = min(R_CACHE, ng)

    nc = bacc.Bacc("TRN2", target_bir_lowering=False, debug=False,
                   num_devices=NCORES)
    x_h = nc.dram_tensor("x", [n_pad, CW], f32, kind="ExternalInput")
    bid_h = nc.dram_tensor("bid", [n_pad], f32, kind="ExternalInput")
    iota_h = nc.dram_tensor("iota8", [GPC], f32, kind="ExternalInput")
    w_h = nc.dram_tensor("w", [224], f32, kind="ExternalInput")
    b_h = nc.dram_tensor("b", [128], f32, kind="ExternalInput")
    out_h = nc.dram_tensor("out", [n_pad, C], f32, kind="ExternalOutput")

    x_g = x_h.ap().rearrange("(g k p) c -> g p k c", p=P, k=KSUB)
    out_g = out_h.ap().rearrange("(g k p) c -> g p k c", p=P, k=KSUB)

    with tile.TileContext(nc) as tc:
        with (
            tc.tile_pool(name="const", bufs=1) as cp,
            tc.tile_pool(name="xt", bufs=XT_BUFS) as xp,
            tc.tile_pool(name="xcache", bufs=max(r_cache, 1)) as xcp,
            tc.tile_pool(name="sq", bufs=2) as sqp,
            tc.tile_pool(name="oh", bufs=2) as ohp,
            tc.tile_pool(name="ps1", bufs=2, space="PSUM") as ps1,
            tc.tile_pool(name="ps2", bufs=2, space="PSUM") as ps2,
        ):
            # ---- constants ----
            iota_t = cp.tile([P, GPC], f32, tag="iota_t")
            nc.gpsimd.dma_start(out=iota_t[:], in_=bass.AP(
                tensor=iota_h, offset=0, ap=[[0, P], [1, GPC]]))
            iota_c = cp.tile([GPC, 1], f32, tag="iota_c")
            nc.gpsimd.dma_start(out=iota_c[:], in_=bass.AP(
                tensor=iota_h, offset=0, ap=[[1, GPC], [1, 1]]))
            w_b = cp.tile([GPC, 224], f32, tag="w_b")
            nc.gpsimd.dma_start(out=w_b[:], in_=bass.AP(
                tensor=w_h, offset=0, ap=[[0, GPC], [1, 224]]))
            bias_b = cp.tile([GPC, 128], f32, tag="bias_b")
            nc.gpsimd.dma_start(out=bias_b[:], in_=bass.AP(
                tensor=b_h, offset=0, ap=[[0, GPC], [1, 128]]))

            acc_sq = cp.tile([GPC, C], f32, tag="acc_sq")
            acc_x = cp.tile([GPC, 128], f32, tag="acc_x")
            acc_c = cp.tile([GPC, 1], f32, tag="acc_c")
            nc.vector.memset(acc_sq[:], 0.0)
            nc.vector.memset(acc_x[:], 0.0)
            nc.vector.memset(acc_c[:], 0.0)

            # ---- phase 1: segment sums ----
            cached = {}
            for g in range(ng):
                if g >= ng - r_cache:
                    xt = xcp.tile([P, KSUB, CW], f32, tag="xc")
                    cached[g] = xt
                else:
                    xt = xp.tile([P, KSUB, CW], f32, tag="xa")
                nc.sync.dma_start(out=xt[:], in_=x_g[g])

                sq = sqp.tile([P, KSUB, C], f32, tag="sq")
                nc.scalar.activation(out=sq[:], in_=xt[:, :, 0:C],
                                     func=Act.Square)

                # one-hot [P, KSUB, GPC]: (bid == iota) per row
                oh = ohp.tile([P, KSUB, GPC], f32, tag="oh")
                bid_ap = xt[:, :, C:C + 1]
                in0 = bass.AP(tensor=bid_ap.tensor, offset=bid_ap.offset,
                              ap=[bid_ap.ap[0], bid_ap.ap[1], [0, GPC]])
                it = iota_t[:]
                in1 = bass.AP(tensor=it.tensor, offset=it.offset,
                              ap=[it.ap[0], [0, KSUB], it.ap[1]])
                nc.vector.tensor_tensor(out=oh[:], in0=in0, in1=in1,
                                        op=Alu.is_equal)

                p_sq = ps1.tile([GPC, C], f32, tag="p_sq")
                p_x = ps1.tile([GPC, 128], f32, tag="p_x")
                p_c = ps1.tile([GPC, 1], f32, tag="p_c")
                for k in range(KSUB):
                    st, sp = (k == 0), (k == KSUB - 1)
                    lhsT = oh[:, k, :]
                    nc.tensor.matmul(out=p_sq[:], lhsT=lhsT, rhs=sq[:, k, :],
                                     start=st, stop=sp)
                    nc.tensor.matmul(out=p_x[:], lhsT=lhsT,
                                     rhs=xt[:, k, 0:128], start=st, stop=sp)
                    nc.tensor.matmul(out=p_c[:], lhsT=lhsT,
                                     rhs=xt[:, k, C + 1:C + 2],
                                     start=st, stop=sp)
                nc.vector.tensor_tensor(out=acc_sq[:], in0=acc_sq[:],
                                        in1=p_sq[:], op=Alu.add)
                nc.vector.tensor_tensor(out=acc_x[:], in0=acc_x[:],
                                        in1=p_x[:], op=Alu.add)
                nc.vector.tensor_tensor(out=acc_c[:], in0=acc_c[:],
                                        in1=p_c[:], op=Alu.add)

            # ---- params: A (scale) and B (shift) per (graph, channel) ----
            invc = cp.tile([GPC, 1], f32, tag="invc")
            nc.vector.tensor_scalar_max(out=invc[:], in0=acc_c[:], scalar1=1.0)
            nc.vector.reciprocal(out=invc[:], in_=invc[:])

            esq = cp.tile([GPC, C], f32, tag="esq")
            nc.vector.tensor_scalar_mul(out=esq[:], in0=acc_sq[:],
                                        scalar1=invc[:])
            m_t = cp.tile([GPC, 128], f32, tag="m_t")
            nc.vector.tensor_scalar_mul(out=m_t[:], in0=acc_x[:],
                                        scalar1=invc[:])

            var = cp.tile([GPC, 128], f32, tag="var")
            nc.vector.tensor_tensor(out=var[:], in0=m_t[:], in1=m_t[:],
                                    op=Alu.mult)
            nc.vector.tensor_tensor(out=var[:], in0=esq[:, 0:128], in1=var[:],
                                    op=Alu.subtract)
            e3 = cp.tile([GPC, 64], f32, tag="e3")
            nc.vector.tensor_reduce(out=e3[:],
                                    in_=esq[:, 128:320].rearrange(
                                        "p (j d) -> p j d", d=3),
                                    axis=mybir.AxisListType.X, op=Alu.add)
            e5 = cp.tile([GPC, 32], f32, tag="e5")
            nc.vector.tensor_reduce(out=e5[:],
                                    in_=esq[:, 320:480].rearrange(
                                        "p (j d) -> p j d", d=5),
                                    axis=mybir.AxisListType.X, op=Alu.add)

            # rstd = 1/sqrt(fn + eps); Rsqrt on ACT is banned for accuracy
            eps_t = cp.tile([GPC, 1], f32, tag="eps_t")
            nc.vector.memset(eps_t[:], EPS)
            nc.scalar.activation(out=var[:], in_=var[:], func=Act.Sqrt,
                                 bias=eps_t[:], scale=1.0)
            nc.vector.reciprocal(out=var[:], in_=var[:])
            nc.scalar.activation(out=e3[:], in_=e3[:], func=Act.Sqrt,
                                 bias=eps_t[:], scale=1.0 / 3.0)
            nc.vector.reciprocal(out=e3[:], in_=e3[:])
            nc.scalar.activation(out=e5[:], in_=e5[:], func=Act.Sqrt,
                                 bias=eps_t[:], scale=1.0 / 5.0)
            nc.vector.reciprocal(out=e5[:], in_=e5[:])

            # params layout: [0:128]=A_s, [128:256]=B_s, [256:320]=A_3,
            # [320:352]=A_5
            params = cp.tile([GPC, 352], f32, tag="params")
            nc.vector.tensor_tensor(out=params[:, 0:128], in0=var[:],
                                    in1=w_b[:, 0:128], op=Alu.mult)
            bm = cp.tile([GPC, 128], f32, tag="bm")
            nc.vector.tensor_tensor(out=bm[:], in0=m_t[:],
                                    in1=params[:, 0:128], op=Alu.mult)
            nc.vector.tensor_tensor(out=params[:, 128:256], in0=bias_b[:],
                                    in1=bm[:], op=Alu.subtract)
            nc.vector.tensor_tensor(out=params[:, 256:320], in0=e3[:],
                                    in1=w_b[:, 128:192], op=Alu.mult)
            nc.vector.tensor_tensor(out=params[:, 320:352], in0=e5[:],
                                    in1=w_b[:, 192:224], op=Alu.mult)

            # ---- phase 2: gather + apply ----
            for g in range(ng):
                if g in cached:
                    xt = cached[g]
                else:
                    xt = xp.tile([P, KSUB, CW], f32, tag="xa")
                    nc.sync.dma_start(out=xt[:], in_=x_g[g])

                bt = ohp.tile([GPC, GROUP], f32, tag="bt")
                nc.gpsimd.dma_start(out=bt[:], in_=bass.AP(
                    tensor=bid_h, offset=g * GROUP,
                    ap=[[0, GPC], [1, GROUP]]))
                ohT = ohp.tile([GPC, GROUP], f32, tag="ohT")
                nc.vector.tensor_scalar(out=ohT[:], in0=bt[:],
                                        scalar1=iota_c[:], scalar2=None,
                                        op0=Alu.is_equal)

                for k in range(KSUB):
                    gp = ps2.tile([P, 352], f32, tag="gp")
                    nc.tensor.matmul(out=gp[:], lhsT=ohT[:, k * P:(k + 1) * P],
                                     rhs=params[:], start=True, stop=True)
                    s0 = xt[:, k, 0:128]
                    nc.vector.tensor_tensor(out=s0, in0=s0, in1=gp[:, 0:128],
                                            op=Alu.mult)
                    nc.vector.tensor_tensor(out=s0, in0=s0, in1=gp[:, 128:256],
                                            op=Alu.add)
                    s3 = xt[:, k, 128:320].rearrange("p (j d) -> p j d", d=3)
                    nc.vector.tensor_tensor(out=s3, in0=s3,
                                            in1=_expand(gp[:, 256:320], 3,
                                                        bass),
                                            op=Alu.mult)
                    s5 = xt[:, k, 320:480].rearrange("p (j d) -> p j d", d=5)
                    nc.vector.tensor_tensor(out=s5, in0=s5,
                                            in1=_expand(gp[:, 320:352], 5,
                                                        bass),
                                            op=Alu.mult)
                nc.sync.dma_start(out=out_g[g], in_=xt[:, :, 0:C])

    nc.compile()
    return nc


def kernel(input, batch_id_tensor, weight, bias, _trace=False):
    from concourse import bass_utils

    x = np.ascontiguousarray(np.asarray(input, dtype=np.float32))
    bid = np.asarray(batch_id_tensor).astype(np.int64)
    w = np.asarray(weight, dtype=np.float32)
    b = np.asarray(bias, dtype=np.float32)
    n = x.shape[0]

    # graph-aligned core boundaries
    edges = np.searchsorted(bid, np.arange(0, G + 1, GPC), side="left")
    rows = np.diff(edges)
    n_pad = max(GROUP, int(-(-rows.max() // GROUP)) * GROUP)

    key = n_pad
    if key not in _prog_cache:
        _prog_cache[key] = _build(n_pad)
    nc = _prog_cache[key]

    iota = np.arange(GPC, dtype=np.float32)
    in_maps = []
    for c in range(NCORES):
        lo, hi = int(edges[c]), int(edges[c + 1])
        nc_rows = hi - lo
        xa = np.empty((n_pad, CW), dtype=np.float32)
        xa[:nc_rows, 0:C] = x[lo:hi]
        xa[:nc_rows, C] = (bid[lo:hi] - c * GPC).astype(np.float32)
        xa[:nc_rows, C + 1] = 1.0
        if nc_rows < n_pad:
            xa[nc_rows:, 0:C] = 0.0
            xa[nc_rows:, C] = GPC  # out-of-range id -> no one-hot match
            xa[nc_rows:, C + 1] = 0.0
        in_maps.append({
            "x": xa,
            "bid": np.ascontiguousarray(xa[:, C]),
            "iota8": iota,
            "w": w,
            "b": b,
        })

    res = bass_utils.run_bass_kernel_spmd(
        nc, in_maps, core_ids=list(range(NCORES)), trace=_trace)

    out = np.empty((n, C), dtype=np.float32)
    for c in range(NCORES):
        lo, hi = int(edges[c]), int(edges[c + 1])
        out[lo:hi] = res.results[c]["out"][:hi - lo]
    if _trace:
        return out, res
    return out


# revision 7
# speedup vs baseline: 1.2478x; 1.2478x over previous
"""Trainium2 Bass kernel for e3nn-style BatchNorm (instance norm over graphs).

Problem: x [200000, 480] f32, irreps 128x0e + 64x1o + 32x2e, batch_id sorted
into 64 graphs, weight [224], bias [128].

Math (per graph g, derived from the reference):
  scalar block (cols 0:128, one col per channel c):
    m[g,c]   = mean_g(x_c)
    var[g,c] = mean_g(x_c^2) - m^2
    A[g,c]   = w_c / sqrt(var + eps);  B[g,c] = bias_c - m*A
    out      = x*A + B
  vector blocks (64 chans x dim 3, 32 chans x dim 5):
    fn[g,j]  = mean_g(mean_d(x^2))  = (1/d) * sum_d mean_g(x_jd^2)
    A[g,j]   = w_j / sqrt(fn + eps);  out = x*A

Sharding: 8 graphs per core (graph-aligned, boundaries via searchsorted on the
host), each core's rows padded to a common N_pad.  All stats are local to the
core -> no collectives.  Two passes over x per core:
  phase 1: one-hot matmul segment sums.  A ones column rides at col 480 (bid
           at col 481) so the squared 481-wide bf16 matmul also produces the
           per-graph counts.  Sum-of-x for the scalar block comes from a
           second skinny bf16 matmul over a bf16 copy of cols 0:128.
  phase 2: per-graph affine params are gathered back to rows with two bf16
           matmuls (params split hi/lo in bf16, accumulated in fp32 PSUM ->
           ~1e-5 accurate), then a fused in-place affine apply, write out.
bf16 single-pass matmuls are ~4x cheaper on the PE than fp32 LOW_HIGH
dual-pass at half stream rate; stats noise from bf16(x^2) is ~1e-4 relative.
The trailing R_CACHE groups stay resident in SBUF between the phases to skip
part of the second read.
"""

import sys

if "/opt/trn_rl_repo" not in sys.path:
    sys.path.insert(0, "/opt/trn_rl_repo")

import numpy as np

P = 128          # partitions / rows per subtile
KSUB = 8         # subtiles per group
GROUP = P * KSUB # rows per group (1024)
C = 480          # data columns
CW = C + 2       # + ones col (C) + local graph id col (C+1)
NCORES = 8
G = 64           # total graphs
GPC = G // NCORES  # graphs per core
EPS = 1e-5
R_CACHE = 5      # trailing groups kept resident in SBUF between phases
XT_BUFS = 4

_prog_cache = {}


def _expand(ap_in, rep, bass):
    """Append a trailing broadcast dim [0, rep] to an AP."""
    return bass.AP(tensor=ap_in.tensor, offset=ap_in.offset,
                   ap=[*ap_in.ap, [0, rep]])


def _build(n_pad):
    import concourse.bacc as bacc
    import concourse.bass as bass
    import concourse.tile as tile
    from concourse import mybir

    f32 = mybir.dt.float32
    bf16 = mybir.dt.float16  # fp16: same PE rate as bf16, 4x less rounding noise
    Alu = mybir.AluOpType
    Act = mybir.ActivationFunctionType

    ng = n_pad // GROUP
    r_cache = min(R_CACHE, ng)

    nc = bacc.Bacc("TRN2", target_bir_lowering=False, debug=False,
                   num_devices=NCORES)
    x_h = nc.dram_tensor("x", [n_pad, CW], f32, kind="ExternalInput")
    bid_h = nc.dram_tensor("bid", [n_pad], f32, kind="ExternalInput")
    iota_h = nc.dram_tensor("iota8", [GPC], f32, kind="ExternalInput")
    w_h = nc.dram_tensor("w", [224], f32, kind="ExternalInput")
    b_h = nc.dram_tensor("b", [128], f32, kind="ExternalInput")
    out_h = nc.dram_tensor("out", [n_pad, C], f32, kind="ExternalOutput")

    x_g = x_h.ap().rearrange("(g k p) c -> g p k c", p=P, k=KSUB)
    out_g = out_h.ap().rearrange("(g k p) c -> g p k c", p=P, k=KSUB)

    with tile.TileContext(nc) as tc:
        with (
            tc.tile_pool(name="const", bufs=1) as cp,
            tc.tile_pool(name="xt", bufs=XT_BUFS) as xp,
            tc.tile_pool(name="xcache", bufs=max(r_cache, 1)) as xcp,
            tc.tile_pool(name="sq", bufs=2) as sqp,
            tc.tile_pool(name="oh", bufs=2) as ohp,
            tc.tile_pool(name="ps1", bufs=2, space="PSUM") as ps1,
            tc.tile_pool(name="ps2", bufs=2, space="PSUM") as ps2,
        ):
            # ---- constants ----
            iota_t = cp.tile([P, GPC], f32, tag="iota_t")
            nc.gpsimd.dma_start(out=iota_t[:], in_=bass.AP(
                tensor=iota_h, offset=0, ap=[[0, P], [1, GPC]]))
            iota_c = cp.tile([GPC, 1], f32, tag="iota_c")
            nc.gpsimd.dma_start(out=iota_c[:], in_=bass.AP(
                tensor=iota_h, offset=0, ap=[[1, GPC], [1, 1]]))
            w_b = cp.tile([GPC, 224], f32, tag="w_b")
            nc.gpsimd.dma_start(out=w_b[:], in_=bass.AP(
                tensor=w_h, offset=0, ap=[[0, GPC], [1, 224]]))
            bias_b = cp.tile([GPC, 128], f32, tag="bias_b")
            nc.gpsimd.dma_start(out=bias_b[:], in_=bass.AP(
                tensor=b_h, offset=0, ap=[[0, GPC], [1, 128]]))

            acc_sq = cp.tile([GPC, C + 1], f32, tag="acc_sq")
            acc_x = cp.tile([GPC, 128], f32, tag="acc_x")
            nc.vector.memset(acc_sq[:], 0.0)
            nc.vector.memset(acc_x[:], 0.0)

            # ---- phase 1: segment sums ----
            cached = {}
            for g in range(ng):
                if g >= ng - r_cache:
                    xt = xcp.tile([P, KSUB, CW], f32, tag="xc")
                    cached[g] = xt
                else:
                    xt = xp.tile([P, KSUB, CW], f32, tag="xa")
                nc.sync.dma_start(out=xt[:], in_=x_g[g])

                # bf16 squares incl. the ones col (-> counts in col C)
                sq = sqp.tile([P, KSUB, C + 1], bf16, tag="sq")
                nc.scalar.activation(out=sq[:], in_=xt[:, :, 0:C + 1],
                                     func=Act.Square)
                # bf16 copy of the scalar block for sum-of-x
                xbf = sqp.tile([P, KSUB, 128], bf16, tag="xbf")
                nc.vector.tensor_copy(out=xbf[:], in_=xt[:, :, 0:128])

                # one-hot [P, KSUB, GPC]: (bid == iota) per row, bf16
                oh = ohp.tile([P, KSUB, GPC], bf16, tag="oh")
                bid_ap = xt[:, :, C + 1:C + 2]
                in0 = bass.AP(tensor=bid_ap.tensor, offset=bid_ap.offset,
                              ap=[bid_ap.ap[0], bid_ap.ap[1], [0, GPC]])
                it = iota_t[:]
                in1 = bass.AP(tensor=it.tensor, offset=it.offset,
                              ap=[it.ap[0], [0, KSUB], it.ap[1]])
                nc.vector.tensor_tensor(out=oh[:], in0=in0, in1=in1,
                                        op=Alu.is_equal)

                p_sq = ps1.tile([GPC, C + 1], f32, tag="p_sq")
                p_x = ps1.tile([GPC, 128], f32, tag="p_x")
                for k in range(KSUB):
                    st, sp = (k == 0), (k == KSUB - 1)
                    lhsT = oh[:, k, :]
                    nc.tensor.matmul(out=p_sq[:], lhsT=lhsT, rhs=sq[:, k, :],
                                     start=st, stop=sp)
                    nc.tensor.matmul(out=p_x[:], lhsT=lhsT, rhs=xbf[:, k, :],
                                     start=st, stop=sp)
                nc.vector.tensor_tensor(out=acc_sq[:], in0=acc_sq[:],
                                        in1=p_sq[:], op=Alu.add)
                nc.vector.tensor_tensor(out=acc_x[:], in0=acc_x[:],
                                        in1=p_x[:], op=Alu.add)

            # ---- params: A (scale) and B (shift) per (graph, channel) ----
            invc = cp.tile([GPC, 1], f32, tag="invc")
            nc.vector.tensor_scalar_max(out=invc[:], in0=acc_sq[:, C:C + 1],
                                        scalar1=1.0)
            nc.vector.reciprocal(out=invc[:], in_=invc[:])

            esq = cp.tile([GPC, C], f32, tag="esq")
            nc.vector.tensor_scalar_mul(out=esq[:], in0=acc_sq[:, 0:C],
                                        scalar1=invc[:])
            m_t = cp.tile([GPC, 128], f32, tag="m_t")
            nc.vector.tensor_scalar_mul(out=m_t[:], in0=acc_x[:],
                                        scalar1=invc[:])

            var = cp.tile([GPC, 128], f32, tag="var")
            nc.vector.tensor_tensor(out=var[:], in0=m_t[:], in1=m_t[:],
                                    op=Alu.mult)
            nc.vector.tensor_tensor(out=var[:], in0=esq[:, 0:128], in1=var[:],
                                    op=Alu.subtract)
            e3 = cp.tile([GPC, 64], f32, tag="e3")
            nc.vector.tensor_reduce(out=e3[:],
                                    in_=esq[:, 128:320].rearrange(
                                        "p (j d) -> p j d", d=3),
                                    axis=mybir.AxisListType.X, op=Alu.add)
            e5 = cp.tile([GPC, 32], f32, tag="e5")
            nc.vector.tensor_reduce(out=e5[:],
                                    in_=esq[:, 320:480].rearrange(
                                        "p (j d) -> p j d", d=5),
                                    axis=mybir.AxisListType.X, op=Alu.add)

            # rstd = 1/sqrt(fn + eps); Rsqrt on ACT is banned for accuracy
            eps_t = cp.tile([GPC, 1], f32, tag="eps_t")
            nc.vector.memset(eps_t[:], EPS)
            nc.scalar.activation(out=var[:], in_=var[:], func=Act.Sqrt,
                                 bias=eps_t[:], scale=1.0)
            nc.vector.reciprocal(out=var[:], in_=var[:])
            nc.scalar.activation(out=e3[:], in_=e3[:], func=Act.Sqrt,
                                 bias=eps_t[:], scale=1.0 / 3.0)
            nc.vector.reciprocal(out=e3[:], in_=e3[:])
            nc.scalar.activation(out=e5[:], in_=e5[:], func=Act.Sqrt,
                                 bias=eps_t[:], scale=1.0 / 5.0)
            nc.vector.reciprocal(out=e5[:], in_=e5[:])

            # params layout: [0:128]=A_s, [128:256]=B_s, [256:320]=A_3,
            # [320:352]=A_5
            params = cp.tile([GPC, 352], f32, tag="params")
            nc.vector.tensor_tensor(out=params[:, 0:128], in0=var[:],
                                    in1=w_b[:, 0:128], op=Alu.mult)
            bm = cp.tile([GPC, 128], f32, tag="bm")
            nc.vector.tensor_tensor(out=bm[:], in0=m_t[:],
                                    in1=params[:, 0:128], op=Alu.mult)
            nc.vector.tensor_tensor(out=params[:, 128:256], in0=bias_b[:],
                                    in1=bm[:], op=Alu.subtract)
            nc.vector.tensor_tensor(out=params[:, 256:320], in0=e3[:],
                                    in1=w_b[:, 128:192], op=Alu.mult)
            nc.vector.tensor_tensor(out=params[:, 320:352], in0=e5[:],
                                    in1=w_b[:, 192:224], op=Alu.mult)

            # hi/lo bf16 split (gathered via two bf16 matmuls, fp32 PSUM)
            par_h = cp.tile([GPC, 352], bf16, tag="par_h")
            nc.vector.tensor_copy(out=par_h[:], in_=params[:])
            ph32 = cp.tile([GPC, 352], f32, tag="ph32")
            nc.vector.tensor_copy(out=ph32[:], in_=par_h[:])
            par_l = cp.tile([GPC, 352], bf16, tag="par_l")
            nc.vector.tensor_tensor(out=par_l[:], in0=params[:], in1=ph32[:],
                                    op=Alu.subtract)

            # ---- phase 2: gather + apply ----
            for g in range(ng):
                if g in cached:
                    xt = cached[g]
                else:
                    xt = xp.tile([P, KSUB, CW], f32, tag="xa")
                    nc.sync.dma_start(out=xt[:], in_=x_g[g])

                bt = ohp.tile([GPC, GROUP], f32, tag="bt")
                nc.gpsimd.dma_start(out=bt[:], in_=bass.AP(
                    tensor=bid_h, offset=g * GROUP,
                    ap=[[0, GPC], [1, GROUP]]))
                ohT = ohp.tile([GPC, GROUP], bf16, tag="ohT")
                nc.vector.tensor_scalar(out=ohT[:], in0=bt[:],
                                        scalar1=iota_c[:], scalar2=None,
                                        op0=Alu.is_equal)

                for k in range(KSUB):
                    gp = ps2.tile([P, 352], f32, tag="gp")
                    lhsT = ohT[:, k * P:(k + 1) * P]
                    nc.tensor.matmul(out=gp[:], lhsT=lhsT, rhs=par_h[:],
                                     start=True, stop=False)
                    nc.tensor.matmul(out=gp[:], lhsT=lhsT, rhs=par_l[:],
                                     start=False, stop=True)
                    s0 = xt[:, k, 0:128]
                    nc.vector.tensor_tensor(out=s0, in0=s0, in1=gp[:, 0:128],
                                            op=Alu.mult)
                    nc.vector.tensor_tensor(out=s0, in0=s0, in1=gp[:, 128:256],
                                            op=Alu.add)
                    s3 = xt[:, k, 128:320].rearrange("p (j d) -> p j d", d=3)
                    nc.vector.tensor_tensor(out=s3, in0=s3,
                                            in1=_expand(gp[:, 256:320], 3,
                                                        bass),
                                            op=Alu.mult)
                    s5 = xt[:, k, 320:480].rearrange("p (j d) -> p j d", d=5)
                    nc.vector.tensor_tensor(out=s5, in0=s5,
                                            in1=_expand(gp[:, 320:352], 5,
                                                        bass),
                                            op=Alu.mult)
                nc.sync.dma_start(out=out_g[g], in_=xt[:, :, 0:C])

    nc.compile()
    return nc


def kernel(input, batch_id_tensor, weight, bias, _trace=False):
    from concourse import bass_utils

    x = np.ascontiguousarray(np.asarray(input, dtype=np.float32))
    bid = np.asarray(batch_id_tensor).astype(np.int64)
    w = np.asarray(weight, dtype=np.float32)
    b = np.asarray(bias, dtype=np.float32)
    n = x.shape[0]

    # graph-aligned core boundaries
    edges = np.searchsorted(bid, np.arange(0, G + 1, GPC), side="left")
    rows = np.diff(edges)
    n_pad = max(GROUP, int(-(-rows.max() // GROUP)) * GROUP)

    key = n_pad
    if key not in _prog_cache:
        _prog_cache[key] = _build(n_pad)
    nc = _prog_cache[key]

    iota = np.arange(GPC, dtype=np.float32)
    in_maps = []
    for c in range(NCORES):
        lo, hi = int(edges[c]), int(edges[c + 1])
        nc_rows = hi - lo
        xa = np.empty((n_pad, CW), dtype=np.float32)
        xa[:nc_rows, 0:C] = x[lo:hi]
        xa[:nc_rows, C] = 1.0
        xa[:nc_rows, C + 1] = (bid[lo:hi] - c * GPC).astype(np.float32)
        if nc_rows < n_pad:
            xa[nc_rows:, 0:C] = 0.0
            xa[nc_rows:, C] = 0.0
            xa[nc_rows:, C + 1] = GPC  # out-of-range id -> no one-hot match
        in_maps.append({
            "x": xa,
            "bid": np.ascontiguousarray(xa[:, C + 1]),
            "iota8": iota,
            "w": w,
            "b": b,
        })

    res = bass_utils.run_bass_kernel_spmd(
        nc, in_maps, core_ids=list(range(NCORES)), trace=_trace)

    out = np.empty((n, C), dtype=np.float32)
    for c in range(NCORES):
        lo, hi = int(edges[c]), int(edges[c + 1])
        out[lo:hi] = res.results[c]["out"][:hi - lo]
    if _trace:
        return out, res
    return out
